# revision 1
# baseline (speedup 1.0000x reference)
"""Anti-alias filter (grouped conv -> BN -> softmax -> 9-tap weighted sum)
as a data-parallel Bass/Tile kernel on 8 TRN2 NeuronCores (batch sharded,
2 images per core, no cross-core communication).

Per-core pipeline (chunks of 16 output rows, first chunk split 8+8 to
shorten the ramp):
  - input staged f32 by DMA, converted to fp16 + reflect-padded on ACT.
  - conv channel-partitioned: 18 zero-padded block-diagonal fp16 matmuls
    (2 channel halves x 9 taps) accumulate sigma [72, 512px] in PSUM.
  - BN folded in on the host: scale into the conv weights, shift into the
    exp bias of one fused ACT activation (exp(sig + b) -> E, bf16).
  - E transposed to pixel-partition via PE matmul whose "identity" carries
    an extra all-ones column, so the softmax denominator Z arrives free as
    output column 72. DVE: 1/Z and E_T/Z (fp16), then ACT duplicates each
    weight into adjacent pairs.
  - tap-sum pixel-partitioned on DVE: 9 x (TT-mul + TT-add) fp16 at 2x
    mode, [128w, rows*256c] per op; the per-group weight broadcast is a
    stride-0 AP over the pair-duplicated weights so the innermost dim
    stays step-(+1) and the 2x perf mode engages. x is pre-transposed by
    PE into three dx-shifted pixel-partition copies.
  - PE transposes the fp16 accumulator back to channel-partition; fp16
    DMA out (upcast to f32 on the host).
The DVE tap-sum is the throughput wall (~95% DVE occupancy): 151M
element-ops/core at 2 elem/lane/cycle is the hardware floor for a
per-pixel-weighted 9-tap gather, which no TRN2 engine can express as a
dense matmul.
"""

import os
import sys
from contextlib import ExitStack

import numpy as np

for _p in ("/opt/trn_rl_repo",):
    if os.path.isdir(_p) and _p not in sys.path:
        sys.path.append(_p)

import concourse.bass as bass  # noqa: E402
import concourse.tile as tile  # noqa: E402
from concourse import bacc, mybir  # noqa: E402
from concourse.bass_utils import run_bass_kernel_spmd  # noqa: E402

F32 = mybir.dt.float32
F32R = mybir.dt.float32r
F16 = mybir.dt.float16
BF16 = mybir.dt.bfloat16

N_CORES = 8
N_FULL, C, H, W = 16, 256, 128, 128
IMG_PER_CORE = N_FULL // N_CORES
G = 8
KK = 9  # 3x3 taps
OCH = G * KK  # 72
BN_EPS = 1e-5
CHUNK = 16  # output rows per pipeline chunk
HALO = CHUNK + 2


def _build_kernel_body(ctx: ExitStack, tc: tile.TileContext, out_d, x_d, wq_d,
                       eb_d, idf16_d, idbf16_d, n_img: int, h_img: int):
    nc = tc.nc
    n_chunk = h_img // CHUNK

    consts = ctx.enter_context(tc.tile_pool(name="consts", bufs=1))
    xq_pool = ctx.enter_context(tc.tile_pool(name="xq", bufs=3))
    xp_pool = ctx.enter_context(tc.tile_pool(name="xp", bufs=2))
    xt_pool = ctx.enter_context(tc.tile_pool(name="xt", bufs=2))
    e_pool = ctx.enter_context(tc.tile_pool(name="e", bufs=3))
    e2_pool = ctx.enter_context(tc.tile_pool(name="e2", bufs=3))
    acc_pool = ctx.enter_context(tc.tile_pool(name="acc", bufs=2))
    tmp_pool = ctx.enter_context(tc.tile_pool(name="tmp", bufs=2))
    ost_pool = ctx.enter_context(tc.tile_pool(name="ost", bufs=2))
    small = ctx.enter_context(tc.tile_pool(name="small", bufs=8))

    psig = ctx.enter_context(tc.tile_pool(name="psig", bufs=2, space="PSUM"))
    pet = ctx.enter_context(tc.tile_pool(name="pet", bufs=2, space="PSUM"))
    pxt = ctx.enter_context(tc.tile_pool(name="pxt", bufs=2, space="PSUM"))
    pot = ctx.enter_context(tc.tile_pool(name="pot", bufs=2, space="PSUM"))

    # constants
    w_sb = consts.tile([128, 2, KK, OCH], F16)
    nc.sync.dma_start(w_sb[:], wq_d[:])
    eb_sb = consts.tile([OCH, 1], F32)
    nc.sync.dma_start(eb_sb[:], eb_d[:])
    idf16 = consts.tile([128, 128], F16)
    nc.sync.dma_start(idf16[:], idf16_d[:])
    idbf16 = consts.tile([128, 128], BF16)
    nc.sync.dma_start(idbf16[:], idbf16_d[:])

    # chunk schedule: split the first chunk to shorten the pipeline ramp
    sched = []
    for img in range(n_img):
        r = 0
        for rows in ([8, 8] + [CHUNK] * ((h_img - 16) // CHUNK) if h_img >= 32
                     else [CHUNK] * (h_img // CHUNK)):
            sched.append((img, r, rows))
            r += rows
    for img, r0, rows in sched:
        if True:
            halo = rows + 2
            # ---------------- input staging
            # xp [128c, 2half, 18, 130] fp16; slot s = padded row r0+s,
            # col j = padded col j.  DMA lands f32 in xq; ACT converts.
            xp = xp_pool.tile([128, 2, halo, 130], F16, tag="xp")
            lo = r0 - 1
            dlo = max(lo, 0)
            dhi = min(r0 + rows + 1, h_img)
            s0 = dlo - lo
            nrows = dhi - dlo
            for half in range(2):
                xq = xq_pool.tile([128, halo, 128], F32, tag="xq")
                nc.sync.dma_start(
                    xq[:, s0:s0 + nrows, :],
                    x_d[img, half * 128:(half + 1) * 128, dlo:dhi, :])
                nc.scalar.copy(xp[:, half, s0:s0 + nrows, 1:129],
                               xq[:, s0:s0 + nrows, :])
            # column reflect: padded col0 = x col1 = sbuf col2 ; col129 = col127
            nc.scalar.copy(xp[:, :, s0:s0 + nrows, 0],
                           xp[:, :, s0:s0 + nrows, 2])
            nc.scalar.copy(xp[:, :, s0:s0 + nrows, 129],
                           xp[:, :, s0:s0 + nrows, 127])
            # row reflect at image edges (padded row 0 = x row1 = slot2, etc.)
            if r0 == 0:
                nc.scalar.copy(xp[:, :, 0, :], xp[:, :, 2, :])
            if r0 + rows == h_img:
                nc.scalar.copy(xp[:, :, halo - 1, :],
                               xp[:, :, halo - 3, :])

            # ---------------- conv + exp: E [72, CHUNK*W] bf16
            E = e_pool.tile([OCH, rows * W], BF16, tag="E")
            for qt in range(rows // 4):
                sig = psig.tile([OCH, 512], F32, tag="sig")
                for half in range(2):
                    for tp in range(KK):
                        dy, dx = tp // 3, tp % 3
                        nc.tensor.matmul(
                            sig[:, :],
                            w_sb[:, half, tp, :],
                            xp[:, half, qt * 4 + dy:qt * 4 + dy + 4,
                               dx:dx + 128],
                            start=(half == 0 and tp == 0),
                            stop=(half == 1 and tp == KK - 1),
                        )
                nc.scalar.activation(
                    E[:, qt * 512:(qt + 1) * 512], sig[:, :],
                    mybir.ActivationFunctionType.Exp,
                    bias=eb_sb[:, 0:1], scale=1.0)

            # ---------------- transpose E, softmax denom, normalized dup weights
            # E2 [128w, CHUNK, 144] fp16 : E2[w, h, (g*9+k)*2+q] = E_T/Z
            # idbf16 carries an extra all-ones column at col 72, so each
            # transpose also emits Z = sum_j E_T[j] as output column 72.
            e2 = e2_pool.tile([128, rows, 2 * OCH], F16, tag="e2")
            for qt in range(rows // 4):
                et = pet.tile([128, 4, OCH + 1], F32, tag="et")
                for hh in range(4):
                    h = qt * 4 + hh
                    nc.tensor.matmul(
                        et[:, hh, :],
                        E[:, h * W:(h + 1) * W],
                        idbf16[0:OCH, 0:OCH + 1],
                        start=True, stop=True)
                rz4 = small.tile([128, 4], F32, tag="rz4")
                nc.vector.reciprocal(rz4[:], et[:, :, OCH])
                # normalize: e2n = E_T / Z  (fp16, <= 1)
                e2n = small.tile([128, 4, OCH], F16, tag="e2n")
                nc.vector.tensor_mul(
                    e2n[:], et[:, :, 0:OCH],
                    rz4[:].unsqueeze(2).broadcast_to((128, 4, OCH)))
                # duplicate into adjacent pairs for 2x-mode broadcast reads
                nc.scalar.copy(
                    e2[:, qt * 4:(qt + 1) * 4, 0:2 * OCH:2], e2n[:])
                nc.scalar.copy(
                    e2[:, qt * 4:(qt + 1) * 4, 1:2 * OCH:2], e2n[:])

            # ---------------- x transposed to pixel-partition, 3 dx variants
            # xt [128w, 3dx, 18, 256c] fp16 ; xt[w, dx, s, c] = xp[c, r0+s, w+dx]
            xt = xt_pool.tile([128, 3, halo, 256], F16, tag="xt")
            for half in range(2):
                for dx in range(3):
                    for rb in range(0, halo, 8):
                        nb = min(8, halo - rb)
                        ptx = pxt.tile([128, 1024], F16, tag="ptx")
                        for j in range(nb):
                            nc.tensor.transpose(
                                ptx[:, j * 128:(j + 1) * 128],
                                xp[:, half, rb + j, dx:dx + 128],
                                idf16[:, :])
                        nc.scalar.copy(
                            xt[:, dx, rb:rb + nb, half * 128:(half + 1) * 128],
                            ptx[:, 0:nb * 128].rearrange(
                                "p (h c) -> p h c", h=nb))

            # ---------------- tap-sum on DVE (pixel-partitioned, fp16 2x)
            acc = acc_pool.tile([128, rows, 256], F16, tag="acc")
            tmp = tmp_pool.tile([128, rows, 256], F16, tag="tmp")
            accv = acc[:].rearrange("p h (g s q) -> p h g s q", g=G, q=2)
            tmpv = tmp[:].rearrange("p h (g s q) -> p h g s q", g=G, q=2)
            for tp in range(KK):
                dy, dx = tp // 3, tp % 3
                in0 = xt[:, dx, dy:dy + rows, :].rearrange(
                    "p h (g s q) -> p h g s q", g=G, q=2)
                in1 = (e2[:]
                       .rearrange("p h (g n) -> p h g n", g=G)
                       [:, :, :, 2 * tp:2 * tp + 2]
                       .unsqueeze(3)
                       .broadcast_to((128, rows, G, 16, 2)))
                dst = accv if tp == 0 else tmpv
                nc.vector.tensor_mul(dst, in0, in1)
                if tp > 0:
                    nc.vector.tensor_add(acc[:], acc[:], tmp[:])

            # ---------------- transpose back + output DMA (fp16 out)
            for half in range(2):
                ost = ost_pool.tile([128, rows, 128], F16, tag="ost")
                for rb in range(0, rows, 8):
                    pto = pot.tile([128, 1024], F16, tag="pto")
                    for j in range(8):
                        nc.tensor.transpose(
                            pto[:, j * 128:(j + 1) * 128],
                            acc[:, rb + j, half * 128:(half + 1) * 128],
                            idf16[:, :])
                    nc.scalar.copy(
                        ost[:, rb:rb + 8, :],
                        pto[:].rearrange("p (h c) -> p h c", h=8))
                nc.sync.dma_start(
                    out_d[img, half * 128:(half + 1) * 128, r0:r0 + rows, :],
                    ost[:])


def build_nc(n_img=IMG_PER_CORE, h_img=H):
    nc = bacc.Bacc("TRN2", target_bir_lowering=False, debug=False,
                   num_devices=N_CORES)
    x_d = nc.dram_tensor("x", (n_img, C, h_img, W), F32, kind="ExternalInput")
    wq_d = nc.dram_tensor("wq", (128, 2, KK, OCH), F16, kind="ExternalInput")
    eb_d = nc.dram_tensor("ebias", (OCH, 1), F32, kind="ExternalInput")
    idf16_d = nc.dram_tensor("idf16", (128, 128), F16, kind="ExternalInput")
    idbf16_d = nc.dram_tensor("idbf16", (128, 128), BF16, kind="ExternalInput")
    out_d = nc.dram_tensor("out", (n_img, C, h_img, W), F16,
                           kind="ExternalOutput")
    with tile.TileContext(nc) as tc:
        with ExitStack() as ctx:
            _build_kernel_body(ctx, tc, out_d.ap(), x_d.ap(), wq_d.ap(),
                               eb_d.ap(), idf16_d.ap(),
                               idbf16_d.ap(), n_img, h_img)
    nc.compile()
    return nc


def prep_params(conv_w, gamma, beta, running_mean, running_var):
    """Fold BN scale into conv weights; build block-diag lhsT + exp bias."""
    scale = (gamma / np.sqrt(running_var + BN_EPS)).astype(np.float64)
    ebias = (beta - running_mean * scale).astype(np.float32).reshape(OCH, 1)
    w_bn = conv_w.astype(np.float64) * scale[:, None, None, None]
    # wq[c_local, half, tap, o] — zero-padded block-diagonal lhsT per half
    wq = np.zeros((128, 2, KK, OCH), dtype=np.float32)
    for o in range(OCH):
        g = o // KK
        half = g // 4
        for ci in range(C // G):
            c_loc = (g % 4) * 32 + ci
            for tp in range(KK):
                wq[c_loc, half, tp, o] = w_bn[o, ci, tp // 3, tp % 3]
    return wq, ebias


_NC_CACHE = {}


def _get_nc(key, n_img, h_img):
    if key not in _NC_CACHE:
        _NC_CACHE[key] = build_nc(n_img, h_img)
    return _NC_CACHE[key]


def make_in_maps(x, conv_w, gamma, beta, running_mean, running_var,
                 n_cores=N_CORES):
    import ml_dtypes
    wq, ebias = prep_params(conv_w, gamma, beta, running_mean, running_var)
    ident = np.eye(128, dtype=np.float32)
    # idbf16: identity plus an all-ones column at col 72 — the E-transpose
    # then emits the softmax denominator Z as its 73rd output column.
    identz = ident.copy()
    identz[0:OCH, OCH] = 1.0
    base = {
        "wq": wq.astype(np.float16),
        "ebias": ebias,
        "idf16": ident.astype(np.float16),
        "idbf16": identz.astype(ml_dtypes.bfloat16),
    }
    per = x.shape[0] // n_cores
    return [dict(base, x=np.ascontiguousarray(x[i * per:(i + 1) * per]))
            for i in range(n_cores)]


def kernel(x, conv_w, gamma, beta, running_mean, running_var):
    x = np.asarray(x, dtype=np.float32)
    conv_w = np.asarray(conv_w, dtype=np.float32)
    gamma = np.asarray(gamma, dtype=np.float32)
    beta = np.asarray(beta, dtype=np.float32)
    running_mean = np.asarray(running_mean, dtype=np.float32)
    running_var = np.asarray(running_var, dtype=np.float32)

    in_maps = make_in_maps(x, conv_w, gamma, beta, running_mean, running_var)
    nc = _get_nc("full", IMG_PER_CORE, H)
    res = run_bass_kernel_spmd(nc, in_maps, core_ids=list(range(N_CORES)))
    out = np.concatenate([r["out"] for r in res.results], axis=0)
    return out.astype(np.float32)

